# revision 4
# baseline (speedup 1.0000x reference)
"""Trainium2 Bass kernel for nn_ARRPSRT_5660766896388 (moe_routing).

Reference block: reread cross-attn(bank) -> causal GQA self-attn(RoPE)
-> gated cross-attn(m_0) -> beta-scaled 3-expert SwiGLU FFN mixture.

Sharding: 8 cores = 2 batches x 4 sequence chunks of 512 tokens.
Each core owns 512 tokens end-to-end.  K/V for self-attn and memory
cross-attn are projected locally and AllGather'd within each 4-core
batch group (one merged collective per stage).  All activations are
kept transposed on device (features on partitions, tokens on free dim)
so no on-device transposes are needed.  Matmuls run in bf16 with fp32
PSUM accumulation; the residual stream stays fp32.

Host-side folding (numpy, no tensor compute):
  rn_w->rq, sn_w->sq/sk/sv, mn_w->mq, n2_w->wg/wu,
  sigmoid(mem_gate)->mo, expert_weights*beta->wd (per batch).
"""
import numpy as np
import ml_dtypes

import concourse.bass as bass
import concourse.mybir as mybir
import concourse.tile as tile
from concourse.bass_utils import run_bass_kernel_spmd

F32 = mybir.dt.float32
BF16 = mybir.dt.bfloat16
BF = ml_dtypes.bfloat16

B, L, D = 2, 2048, 1024
H, KV, DH = 16, 4, 64
E, FF = 3, 3072
M = 72
EPS = 1e-5
T = 512            # tokens per core
DC = D // 128      # 8 feature chunks
NJ = L // 128      # 16 key tiles
GROUPS = [[0, 1, 2, 3], [4, 5, 6, 7]]
SCALE = float(1.0 / np.sqrt(DH))
# S2 head permutation: position p holds head S2_ORDER[p]; guarantees the
# kv-group row parity of the head matches p's row parity within q tiles.
_EV = [0, 1, 2, 3, 8, 9, 10, 11]
_OD = [4, 5, 6, 7, 12, 13, 14, 15]
S2_ORDER = [x for i in range(8) for x in (_EV[i], _OD[i])]

_SPLIT_CTR = [0]


def _split_sync_waits(nc):
    """This walrus build allows at most ONE sem-wait per instruction; hoist
    extras onto NoOps inserted just before the carrying instruction."""
    for f in nc.m.functions:
        for blk in f.blocks:
            out = []
            for inst in blk.instructions:
                si = inst.sync_info
                waits = list(si.on_wait) if si is not None else []
                if len(waits) > 1:
                    for w in waits[:-1]:
                        _SPLIT_CTR[0] += 1
                        out.append(
                            mybir.InstNoOp(
                                name=f"splitw-{_SPLIT_CTR[0]}",
                                engine=inst.engine,
                                sync_info=mybir.SyncInfo(on_wait=[w], on_update=[]),
                                bass_nofuse=True,
                            )
                        )
                    inst.sync_info = mybir.SyncInfo(
                        on_wait=[waits[-1]], on_update=list(si.on_update)
                    )
                out.append(inst)
            blk.instructions = out


def _build():
    import contextlib

    nc = bass.Bass()
    P = {}

    def inp(name, shape, dt=BF16):
        P[name] = nc.declare_dram_parameter(name, list(shape), dt, isOutput=False)
        return P[name]

    inp("xt", [D, T], F32)
    inp("m0t", [D, T])
    inp("bankt", [D, M])
    inp("cost", [DH // 2, T], F32)
    inp("sint", [DH // 2, T], F32)
    inp("mask", [L, T])
    for w in ("rq", "rk", "rv", "ro", "sq", "so", "mq", "mk", "mv", "mo"):
        inp(w, [D, D])
    inp("sk", [D, KV * DH])
    inp("sv", [D, KV * DH])
    inp("wg", [E, D, FF])
    inp("wu", [E, D, FF])
    inp("wd", [E, FF, D])
    out_ext = nc.declare_dram_parameter("out", [D, T], F32, isOutput=True)

    # collective bounce buffers (flat, bf16): [k | v] per stage
    K2N, V2N = (KV * DH) * T, T * (KV * (DH + 1))          # 131072, 133120
    C2N = K2N + V2N
    K3N, V3N = D * T, T * (H * (DH + 1))                   # 524288, 532480
    C3N = K3N + V3N
    cc2_loc = nc.dram_tensor("cc2_loc", [C2N], BF16)
    cc2_all = nc.dram_tensor("cc2_all", [4 * C2N], BF16)
    cc3_loc = nc.dram_tensor("cc3_loc", [C3N], BF16)
    cc3_all = nc.dram_tensor("cc3_all", [4 * C3N], BF16)

    with tile.TileContext(nc) as tc, contextlib.ExitStack() as ctx:
        sing = ctx.enter_context(tc.tile_pool(name="sing", bufs=1))
        tmp = ctx.enter_context(tc.tile_pool(name="tmp", bufs=3))
        rope_t = ctx.enter_context(tc.tile_pool(name="rope_t", bufs=4))
        pp = ctx.enter_context(tc.tile_pool(name="pp", bufs=2, space="PSUM"))
        pu = ctx.enter_context(tc.tile_pool(name="pu", bufs=2, space="PSUM"))
        ps = ctx.enter_context(tc.tile_pool(name="ps", bufs=2, space="PSUM"))
        po = ctx.enter_context(tc.tile_pool(name="po", bufs=1, space="PSUM"))
        pb = ctx.enter_context(tc.tile_pool(name="pb", bufs=1, space="PSUM"))

        Sqrt = mybir.ActivationFunctionType.Sqrt
        Exp = mybir.ActivationFunctionType.Exp
        Silu = mybir.ActivationFunctionType.Silu
        mm = nc.tensor.matmul

        ones_col = sing.tile([128, 1], BF16, tag="ones_col")
        nc.vector.memset(ones_col[:], 1.0)
        ones_row = sing.tile([1, 128], F32, tag="ones_row")
        nc.vector.memset(ones_row[:], 1.0)
        eps_sb = sing.tile([1, 1], F32, tag="eps")
        nc.vector.memset(eps_sb[:], EPS)

        def load_tiles(ext, n, rows, cols, tagp, dt=BF16, col0=0, pool=None):
            pool = pool or sing
            ts = []
            for i in range(n):
                t = pool.tile([rows, cols], dt, tag=f"{tagp}{i}")
                nc.sync.dma_start(
                    out=t[:], in_=ext[i * rows:(i + 1) * rows, col0:col0 + cols]
                )
                ts.append(t)
            return ts

        # ---------- residual stream (own 512 tokens), fp32 ----------
        xt = load_tiles(P["xt"], DC, 128, T, "xt", F32)

        def rmsnorm(tagp, zpool):
            ss = ps.tile([1, T], F32, tag="ps")
            for dd in range(DC):
                x2 = tmp.tile([128, T], BF16, tag="x2")
                nc.vector.tensor_mul(x2[:], xt[dd][:], xt[dd][:])
                mm(ss[:], ones_col[:], x2[:], start=(dd == 0), stop=(dd == DC - 1))
            rms = tmp.tile([1, T], F32, tag="rms")
            nc.scalar.activation(out=rms[:], in_=ss[:], func=Sqrt,
                                 bias=eps_sb[:], scale=1.0 / D)
            rstd = tmp.tile([1, T], F32, tag="rstd")
            nc.vector.reciprocal(out=rstd[:], in_=rms[:])
            rstd_bc = pb.tile([128, T], F32, tag="pb")
            mm(rstd_bc[:], ones_row[:], rstd[:])
            z = []
            for dd in range(DC):
                zt = zpool.tile([128, T], BF16, tag=f"{tagp}{dd}")
                nc.vector.tensor_mul(zt[:], xt[dd][:], rstd_bc[:])
                z.append(zt)
            return z

        def rope(dst, src_ps, head_rows):
            """dst[head_rows:head_rows+64] = rope(src_ps rows), bf16 out."""
            lo = src_ps[head_rows:head_rows + 32, :]
            hi = src_ps[head_rows + 32:head_rows + 64, :]
            t1 = rope_t.tile([32, T], F32, tag="rp1")
            t2 = rope_t.tile([32, T], F32, tag="rp2")
            nc.vector.tensor_mul(t1[:], lo, cos_sb[:])
            nc.vector.tensor_mul(t2[:], hi, sin_sb[:])
            nc.vector.tensor_sub(dst[head_rows:head_rows + 32, :], t1[:], t2[:])
            t3 = rope_t.tile([32, T], F32, tag="rp3")
            t4 = rope_t.tile([32, T], F32, tag="rp4")
            nc.vector.tensor_mul(t3[:], hi, cos_sb[:])
            nc.vector.tensor_mul(t4[:], lo, sin_sb[:])
            nc.vector.tensor_add(dst[head_rows + 32:head_rows + 64, :], t3[:], t4[:])

        def attend(h, ktile, krow, q_sb, v_sb, vcol, mask_tiles, o_sb, orow):
            """One head: scores over 16 key tiles -> exp -> (mask) -> AV+sum
            -> normalize into o_sb[orow:orow+64]."""
            o_ps = po.tile([DH + 1, T], F32, tag="po")
            for j in range(NJ):
                s_ps = ps.tile([128, T], F32, tag="ps")
                mm(s_ps[:], ktile(j)[krow:krow + DH, (j % 4) * 128:(j % 4 + 1) * 128],
                   q_sb, skip_group_check=True)
                p_sb = tmp.tile([128, T], BF16, tag="p")
                nc.scalar.activation(out=p_sb[:], in_=s_ps[:], func=Exp, scale=SCALE)
                if mask_tiles is not None:
                    nc.vector.tensor_mul(p_sb[:], p_sb[:], mask_tiles[j][:])
                mm(o_ps[:], v_sb(j)[:, vcol:vcol + DH + 1], p_sb[:],
                   start=(j == 0), stop=(j == NJ - 1), skip_group_check=True)
            rec = tmp.tile([1, T], F32, tag="rec")
            nc.vector.reciprocal(out=rec[:], in_=o_ps[DH:DH + 1, :])
            o_u = tmp.tile([DH, T], BF16, tag="o_u")
            nc.scalar.copy(out=o_u[:], in_=o_ps[0:DH, :])
            rec_bc = pb.tile([DH, T], F32, tag="pb")
            mm(rec_bc[:], ones_row[:, 0:DH], rec[:])
            nc.vector.tensor_mul(o_sb[orow:orow + DH, :], o_u[:], rec_bc[:])

        def out_proj(wo_tiles, o_tiles):
            for dd in range(DC):
                d_ps = pp.tile([128, T], F32, tag="pp")
                for hp in range(DC):
                    mm(d_ps[:], wo_tiles[hp][:, dd * 128:(dd + 1) * 128],
                       o_tiles[hp][:], start=(hp == 0), stop=(hp == DC - 1))
                nc.vector.tensor_add(xt[dd][:], xt[dd][:], d_ps[:])

        # ---------- S3 local K/V from m0 + AllGather (issued early) ----------
        with tc.tile_pool(name="s3w", bufs=1) as s3w, \
             tc.tile_pool(name="s3l", bufs=1) as s3l:
            m0t = load_tiles(P["m0t"], DC, 128, T, "m0t", pool=s3w)
            mk = load_tiles(P["mk"], DC, 128, D, "mk", pool=s3w)
            mv = load_tiles(P["mv"], DC, 128, D, "mv", pool=s3w)
            for hp in range(DC):
                k_ps = pp.tile([128, T], F32, tag="pp")
                for dd in range(DC):
                    mm(k_ps[:], mk[dd][:, hp * 128:(hp + 1) * 128], m0t[dd][:],
                       start=(dd == 0), stop=(dd == DC - 1))
                k_sb = s3l.tile([128, T], BF16, tag=f"k3l{hp}")
                nc.vector.tensor_copy(out=k_sb[:], in_=k_ps[:])
                nc.sync.dma_start(
                    out=cc3_loc[hp * 128 * T:(hp + 1) * 128 * T].rearrange(
                        "(p t) -> p t", p=128),
                    in_=k_sb[:])
            for j4 in range(4):
                vA = pp.tile([128, T], F32, tag="pp")
                vB = pu.tile([128, T], F32, tag="pu")
                for dd in range(DC):
                    lhs = m0t[dd][:, j4 * 128:(j4 + 1) * 128]
                    mm(vA[:], lhs, mv[dd][:, 0:512],
                       start=(dd == 0), stop=(dd == DC - 1), skip_group_check=True)
                    mm(vB[:], lhs, mv[dd][:, 512:1024],
                       start=(dd == 0), stop=(dd == DC - 1), skip_group_check=True)
                v_sb = s3l.tile([128, H * (DH + 1)], BF16, tag=f"v3l{j4}")
                for h in range(H):
                    src = vA if h < 8 else vB
                    c0 = (h % 8) * DH
                    nc.vector.tensor_copy(
                        out=v_sb[:, h * (DH + 1):h * (DH + 1) + DH],
                        in_=src[:, c0:c0 + DH])
                    nc.vector.memset(
                        v_sb[:, h * (DH + 1) + DH:(h + 1) * (DH + 1)], 1.0)
                nv = H * (DH + 1)
                nc.sync.dma_start(
                    out=cc3_loc[K3N + j4 * 128 * nv:K3N + (j4 + 1) * 128 * nv]
                    .rearrange("(p t) -> p t", p=128),
                    in_=v_sb[:])
            nc.gpsimd.collective_compute(
                "AllGather", mybir.AluOpType.bypass,
                ins=[cc3_loc[:]], outs=[cc3_all[:]], replica_groups=GROUPS)

        # ---------- rope tables ----------
        cos_sb = sing.tile([DH // 2, T], F32, tag="cos")
        nc.sync.dma_start(out=cos_sb[:], in_=P["cost"][:])
        sin_sb = sing.tile([DH // 2, T], F32, tag="sin")
        nc.sync.dma_start(out=sin_sb[:], in_=P["sint"][:])

        # ---------- S1: reread cross-attn over bank ----------
        with tc.tile_pool(name="s1w", bufs=1) as s1w:
            bankt = load_tiles(P["bankt"], DC, 128, M, "bankt", pool=s1w)
            rq = load_tiles(P["rq"], DC, 128, D, "rq", pool=s1w)
            rk = load_tiles(P["rk"], DC, 128, D, "rk", pool=s1w)
            rv = load_tiles(P["rv"], DC, 128, D, "rv", pool=s1w)
            ro = load_tiles(P["ro"], DC, 128, D, "ro", pool=s1w)
            z1 = rmsnorm("z1_", s1w)

            k1 = []
            for hp in range(DC):
                kp = pp.tile([128, M], F32, tag="pp")
                for dd in range(DC):
                    mm(kp[:], rk[dd][:, hp * 128:(hp + 1) * 128], bankt[dd][:],
                       start=(dd == 0), stop=(dd == DC - 1))
                ks = s1w.tile([128, M], BF16, tag=f"k1_{hp}")
                nc.vector.tensor_copy(out=ks[:], in_=kp[:])
                k1.append(ks)

            v1 = s1w.tile([M, H * (DH + 1)], BF16, tag="v1")
            vA = pp.tile([M, T], F32, tag="pp")
            vB = pu.tile([M, T], F32, tag="pu")
            for dd in range(DC):
                mm(vA[:], bankt[dd][:], rv[dd][:, 0:512],
                   start=(dd == 0), stop=(dd == DC - 1), skip_group_check=True)
                mm(vB[:], bankt[dd][:], rv[dd][:, 512:1024],
                   start=(dd == 0), stop=(dd == DC - 1), skip_group_check=True)
            for h in range(H):
                src = vA if h < 8 else vB
                c0 = (h % 8) * DH
                nc.vector.tensor_copy(out=v1[:, h * (DH + 1):h * (DH + 1) + DH],
                                      in_=src[:, c0:c0 + DH])
                nc.vector.memset(v1[:, h * (DH + 1) + DH:(h + 1) * (DH + 1)], 1.0)

            q1 = []
            for hp in range(DC):
                qp = pp.tile([128, T], F32, tag="pp")
                for dd in range(DC):
                    mm(qp[:], rq[dd][:, hp * 128:(hp + 1) * 128], z1[dd][:],
                       start=(dd == 0), stop=(dd == DC - 1))
                qs = s1w.tile([128, T], BF16, tag=f"q1_{hp}")
                nc.vector.tensor_copy(out=qs[:], in_=qp[:])
                q1.append(qs)

            o1 = [s1w.tile([128, T], BF16, tag=f"o1_{hp}", name=f"o1_{hp}")
                  for hp in range(DC)]
            for h in range(H):
                hp, hr = h // 2, (h % 2) * DH
                o_ps = po.tile([DH + 1, T], F32, tag="po")
                s_ps = ps.tile([M, T], F32, tag="ps")
                mm(s_ps[:], k1[hp][hr:hr + DH, :], q1[hp][hr:hr + DH, :])
                p_sb = tmp.tile([M, T], BF16, tag="p")
                nc.scalar.activation(out=p_sb[:], in_=s_ps[:], func=Exp, scale=SCALE)
                mm(o_ps[:], v1[:, h * (DH + 1):(h + 1) * (DH + 1)], p_sb[:])
                rec = tmp.tile([1, T], F32, tag="rec")
                nc.vector.reciprocal(out=rec[:], in_=o_ps[DH:DH + 1, :])
                o_u = tmp.tile([DH, T], BF16, tag="o_u")
                nc.scalar.copy(out=o_u[:], in_=o_ps[0:DH, :])
                rec_bc = pb.tile([DH, T], F32, tag="pb")
                mm(rec_bc[:], ones_row[:, 0:DH], rec[:])
                nc.vector.tensor_mul(o1[hp][hr:hr + DH, :], o_u[:], rec_bc[:])
            out_proj(ro, o1)

        # ---------- S2: causal GQA self-attn ----------
        with tc.tile_pool(name="s2w", bufs=1) as s2w:
            sq = load_tiles(P["sq"], DC, 128, D, "sq", pool=s2w)
            sk = load_tiles(P["sk"], DC, 128, KV * DH, "sk", pool=s2w)
            sv = load_tiles(P["sv"], DC, 128, KV * DH, "sv", pool=s2w)
            so = load_tiles(P["so"], DC, 128, D, "so", pool=s2w)
            mask_t = load_tiles(P["mask"], NJ, 128, T, "mask", pool=s2w)
            z2 = rmsnorm("z2_", s2w)

            # local K (rope) -> cc2_loc
            for gp in range(2):
                kp = pp.tile([128, T], F32, tag="pp")
                for dd in range(DC):
                    mm(kp[:], sk[dd][:, gp * 128:(gp + 1) * 128], z2[dd][:],
                       start=(dd == 0), stop=(dd == DC - 1))
                ksb = s2w.tile([128, T], BF16, tag=f"k2l{gp}")
                rope(ksb, kp, 0)
                rope(ksb, kp, 64)
                nc.sync.dma_start(
                    out=cc2_loc[gp * 128 * T:(gp + 1) * 128 * T].rearrange(
                        "(p t) -> p t", p=128),
                    in_=ksb[:])
            # local V -> cc2_loc
            nv2 = KV * (DH + 1)
            for j4 in range(4):
                vp = pu.tile([128, KV * DH], F32, tag="pu")
                for dd in range(DC):
                    mm(vp[:], z2[dd][:, j4 * 128:(j4 + 1) * 128], sv[dd][:],
                       start=(dd == 0), stop=(dd == DC - 1))
                vsb = s2w.tile([128, nv2], BF16, tag=f"v2l{j4}")
                for g in range(KV):
                    nc.vector.tensor_copy(
                        out=vsb[:, g * (DH + 1):g * (DH + 1) + DH],
                        in_=vp[:, g * DH:(g + 1) * DH])
                    nc.vector.memset(
                        vsb[:, g * (DH + 1) + DH:(g + 1) * (DH + 1)], 1.0)
                nc.sync.dma_start(
                    out=cc2_loc[K2N + j4 * 128 * nv2:K2N + (j4 + 1) * 128 * nv2]
                    .rearrange("(p t) -> p t", p=128),
                    in_=vsb[:])
            nc.gpsimd.collective_compute(
                "AllGather", mybir.AluOpType.bypass,
                ins=[cc2_loc[:]], outs=[cc2_all[:]], replica_groups=GROUPS)

            # Q with rope (overlaps the AllGather)
            q2 = []
            for hp in range(DC):
                qp = pp.tile([128, T], F32, tag="pp")
                for dd in range(DC):
                    mm(qp[:], sq[dd][:, hp * 128:(hp + 1) * 128], z2[dd][:],
                       start=(dd == 0), stop=(dd == DC - 1))
                qs = s2w.tile([128, T], BF16, tag=f"q2_{hp}")
                rope(qs, qp, 0)
                rope(qs, qp, 64)
                q2.append(qs)

            # gathered K/V back to SBUF
            k2g, v2g = [], []
            for r in range(4):
                for gp in range(2):
                    t = s2w.tile([128, T], BF16, tag=f"k2g{r}_{gp}")
                    off = r * C2N + gp * 128 * T
                    nc.sync.dma_start(
                        out=t[:],
                        in_=cc2_all[off:off + 128 * T].rearrange("(p t) -> p t", p=128))
                    k2g.append(t)
            for r in range(4):
                for j4 in range(4):
                    t = s2w.tile([128, nv2], BF16, tag=f"v2g{r}_{j4}")
                    off = r * C2N + K2N + j4 * 128 * nv2
                    nc.sync.dma_start(
                        out=t[:],
                        in_=cc2_all[off:off + 128 * nv2].rearrange("(p t) -> p t", p=128))
                    v2g.append(t)

            o2 = [s2w.tile([128, T], BF16, tag=f"o2_{hp}", name=f"o2_{hp}")
                  for hp in range(DC)]
            # heads permuted host-side (S2_ORDER) so q-row parity == kv-row
            # parity (PE needs lhsT/rhs on the same base partition)
            for p in range(H):
                h = S2_ORDER[p]
                g = h // (H // KV)
                gp2, grow = g // 2, (g % 2) * DH
                assert grow == (p % 2) * DH
                attend(h,
                       lambda j, gp2=gp2: k2g[(j // 4) * 2 + gp2],
                       grow,
                       q2[p // 2][(p % 2) * DH:(p % 2) * DH + DH, :],
                       lambda j: v2g[j],
                       g * (DH + 1),
                       mask_t,
                       o2[p // 2], (p % 2) * DH)
            out_proj(so, o2)

        # ---------- S3: memory cross-attn (gate folded into mo) ----------
        with tc.tile_pool(name="s3a", bufs=1) as s3a:
            z3 = rmsnorm("z3_", s3a)
            q3 = []
            with tc.tile_pool(name="s3q", bufs=1) as s3q:
                mq = load_tiles(P["mq"], DC, 128, D, "mq", pool=s3q)
                for hp in range(DC):
                    qp = pp.tile([128, T], F32, tag="pp")
                    for dd in range(DC):
                        mm(qp[:], mq[dd][:, hp * 128:(hp + 1) * 128], z3[dd][:],
                           start=(dd == 0), stop=(dd == DC - 1))
                    qs = s3a.tile([128, T], BF16, tag=f"q3_{hp}")
                    nc.vector.tensor_copy(out=qs[:], in_=qp[:])
                    q3.append(qs)

            k3g, v3g = [], []
            nv3 = H * (DH + 1)
            for r in range(4):
                for hp in range(DC):
                    t = s3a.tile([128, T], BF16, tag=f"k3g{r}_{hp}")
                    off = r * C3N + hp * 128 * T
                    nc.sync.dma_start(
                        out=t[:],
                        in_=cc3_all[off:off + 128 * T].rearrange("(p t) -> p t", p=128))
                    k3g.append(t)
            for r in range(4):
                for j4 in range(4):
                    t = s3a.tile([128, nv3], BF16, tag=f"v3g{r}_{j4}")
                    off = r * C3N + K3N + j4 * 128 * nv3
                    nc.sync.dma_start(
                        out=t[:],
                        in_=cc3_all[off:off + 128 * nv3].rearrange("(p t) -> p t", p=128))
                    v3g.append(t)

            o3 = [s3a.tile([128, T], BF16, tag=f"o3_{hp}", name=f"o3_{hp}")
                  for hp in range(DC)]
            for h in range(H):
                hp, hr = h // 2, (h % 2) * DH
                attend(h,
                       lambda j, hp=hp: k3g[(j // 4) * DC + hp],
                       hr,
                       q3[hp][hr:hr + DH, :],
                       lambda j: v3g[j],
                       h * (DH + 1),
                       None,
                       o3[hp], hr)
            with tc.tile_pool(name="s3o", bufs=1) as s3o:
                mo = load_tiles(P["mo"], DC, 128, D, "mo", pool=s3o)
                out_proj(mo, o3)

        # ---------- S4: 3-expert SwiGLU FFN (coef folded into wd) ----------
        with tc.tile_pool(name="s4w", bufs=2) as s4w, \
             tc.tile_pool(name="s4a", bufs=1) as s4a:
            z4 = rmsnorm("z4_", s4a)
            NBLK = 3          # 3 column-blocks of 1024 over FF
            FCB = FF // NBLK // 128   # 8 fc per block
            for e in range(E):
                for blk in range(NBLK):
                    c0 = blk * 1024
                    wg_sl = [s4w.tile([128, 1024], BF16, tag=f"wg{dd}",
                                      name=f"wg{e}_{blk}_{dd}") for dd in range(DC)]
                    wu_sl = [s4w.tile([128, 1024], BF16, tag=f"wu{dd}",
                                      name=f"wu{e}_{blk}_{dd}") for dd in range(DC)]
                    for dd in range(DC):
                        nc.sync.dma_start(
                            out=wg_sl[dd][:],
                            in_=P["wg"][e, dd * 128:(dd + 1) * 128, c0:c0 + 1024])
                        nc.sync.dma_start(
                            out=wu_sl[dd][:],
                            in_=P["wu"][e, dd * 128:(dd + 1) * 128, c0:c0 + 1024])
                    wd_sl = [s4w.tile([128, 1024], BF16, tag=f"wd{fc}",
                                      name=f"wd{e}_{blk}_{fc}") for fc in range(FCB)]
                    for fc in range(FCB):
                        nc.sync.dma_start(
                            out=wd_sl[fc][:],
                            in_=P["wd"][e, c0 + fc * 128:c0 + (fc + 1) * 128, :])
                    acts = []
                    for fc in range(FCB):
                        gps = pp.tile([128, T], F32, tag="pp")
                        ups = pu.tile([128, T], F32, tag="pu")
                        for dd in range(DC):
                            mm(gps[:], wg_sl[dd][:, fc * 128:(fc + 1) * 128],
                               z4[dd][:], start=(dd == 0), stop=(dd == DC - 1),
                               skip_group_check=True)
                            mm(ups[:], wu_sl[dd][:, fc * 128:(fc + 1) * 128],
                               z4[dd][:], start=(dd == 0), stop=(dd == DC - 1),
                               skip_group_check=True)
                        sl = tmp.tile([128, T], BF16, tag="sl")
                        nc.scalar.activation(out=sl[:], in_=gps[:], func=Silu)
                        act = s4a.tile([128, T], BF16, tag=f"act{fc}")
                        nc.vector.tensor_mul(act[:], sl[:], ups[:])
                        acts.append(act)
                    for dd in range(DC):
                        d_ps = ps.tile([128, T], F32, tag="ps")
                        for fc in range(FCB):
                            mm(d_ps[:], wd_sl[fc][:, dd * 128:(dd + 1) * 128],
                               acts[fc][:], start=(fc == 0), stop=(fc == FCB - 1))
                        nc.vector.tensor_add(xt[dd][:], xt[dd][:], d_ps[:])

        # ---------- output ----------
        for dd in range(DC):
            nc.sync.dma_start(out=out_ext[dd * 128:(dd + 1) * 128, :], in_=xt[dd][:])

    _split_sync_waits(nc)
    return nc


_NC = None


def _get_nc():
    global _NC
    if _NC is None:
        _NC = _build()
    return _NC


def _prep_in_maps(r, m_0, bank_scratch, rope_cos, rope_sin, expert_weights, beta,
                  params):
    p = params
    sig = 1.0 / (1.0 + np.exp(-np.float32(p["mem_gate"])))
    w = {}
    w["rq"] = (p["rn_w"][:, None] * p["rq"]).astype(BF)
    w["rk"] = np.asarray(p["rk"], np.float32).astype(BF)
    w["rv"] = np.asarray(p["rv"], np.float32).astype(BF)
    w["ro"] = np.asarray(p["ro"], np.float32).astype(BF)
    sq_f = np.asarray(p["sn_w"], np.float32)[:, None] * np.asarray(p["sq"], np.float32)
    so_f = np.asarray(p["so"], np.float32)
    sq_perm = np.empty_like(sq_f)
    so_perm = np.empty_like(so_f)
    for pos, h in enumerate(S2_ORDER):
        sq_perm[:, pos * DH:(pos + 1) * DH] = sq_f[:, h * DH:(h + 1) * DH]
        so_perm[pos * DH:(pos + 1) * DH, :] = so_f[h * DH:(h + 1) * DH, :]
    w["sq"] = sq_perm.astype(BF)
    w["sk"] = (p["sn_w"][:, None] * p["sk"]).astype(BF)
    w["sv"] = (p["sn_w"][:, None] * p["sv"]).astype(BF)
    w["so"] = so_perm.astype(BF)
    w["mq"] = (p["mn_w"][:, None] * p["mq"]).astype(BF)
    w["mk"] = np.asarray(p["mk"], np.float32).astype(BF)
    w["mv"] = np.asarray(p["mv"], np.float32).astype(BF)
    w["mo"] = (sig * np.asarray(p["mo"], np.float32)).astype(BF)
    w["wg"] = (p["n2_w"][None, :, None] * p["wg"]).astype(BF)
    w["wu"] = (p["n2_w"][None, :, None] * p["wu"]).astype(BF)
    coef = np.asarray(expert_weights, np.float32) * np.asarray(beta, np.float32)[None, :]
    wd_b = [(np.asarray(p["wd"], np.float32)
             * coef[b][:, None, None]).astype(BF) for b in range(B)]

    cosT = np.ascontiguousarray(np.asarray(rope_cos, np.float32).T)  # [32, L]
    sinT = np.ascontiguousarray(np.asarray(rope_sin, np.float32).T)

    in_maps = []
    for core in range(8):
        b, c = core // 4, core % 4
        lo, hi = c * T, (c + 1) * T
        keys = np.arange(L)[:, None]
        qs = np.arange(lo, hi)[None, :]
        mask = (keys <= qs).astype(BF)
        im = dict(w)
        im["wd"] = wd_b[b]
        im["xt"] = np.ascontiguousarray(np.asarray(r[b], np.float32)[lo:hi].T)
        im["m0t"] = np.ascontiguousarray(
            np.asarray(m_0[b], np.float32)[lo:hi].T.astype(BF))
        im["bankt"] = np.ascontiguousarray(
            np.asarray(bank_scratch[b], np.float32).T.astype(BF))
        im["cost"] = np.ascontiguousarray(cosT[:, lo:hi])
        im["sint"] = np.ascontiguousarray(sinT[:, lo:hi])
        im["mask"] = mask
        in_maps.append(im)
    return in_maps


def _run(inputs, trace=False, trace_kwargs=None):
    nc = _get_nc()
    in_maps = _prep_in_maps(**inputs)
    res = run_bass_kernel_spmd(nc, in_maps, list(range(8)), trace=trace,
                               **(trace_kwargs or {}))
    out = np.empty((B, L, D), np.float32)
    for core in range(8):
        b, c = core // 4, core % 4
        out[b, c * T:(c + 1) * T, :] = res.results[core]["out"].T
    return out, res


def kernel(**inputs) -> np.ndarray:
    out, _ = _run(inputs, trace=False)
    return out
